# revision 1
# baseline (speedup 1.0000x reference)
"""Trainium2 Bass kernel for nn_Attention_66932770341587 (MEGA-style block).

Contract: kernel(**inputs) takes FULL unsharded inputs (as in setup_inputs),
returns the FULL [8, 2048, 768] output. Internally: pure data-parallel over
batch across 8 NeuronCores; each core computes one batch element in a
feature-major ("T") layout.

Per-core pipeline:
  P1: vproj matmuls (float32r) -> silu (ACT) -> per-column min/max ->
      uint16 quantize with direction folded into the affine sign.
  P2: EMA via 96 tensor_tensor_scan ops + PE diag-matmul combine (+omega) in
      PSUM -> silu -> mx (spilled to DRAM scratch).
  S:  66-stage flip-bitonic sort of uint16 keys on DVE (2x mode).
  P3a: mxproj (u/r/hx) from mx slices, ACT epilogues, spilled to DRAM.
  P3b: per l-block: dequantize sorted keys, t1 = sorted*r, hproj, h, y.
"""

import numpy as np
from contextlib import ExitStack

import concourse.bass as bass
import concourse.mybir as mybir
import concourse.tile as tile
from concourse import bacc, bass_utils

F32 = mybir.dt.float32
F32R = mybir.dt.float32r
U16 = mybir.dt.uint16
AF = mybir.ActivationFunctionType
OP = mybir.AluOpType

D, L, H, N = 768, 2048, 768, 16
G = 6                 # 128-partition d-groups
LB = 512              # l-block for P1/P2/P3a matmuls
LB3 = 256             # l-block for P3b epilogue
NLB = L // LB
NLB3 = L // LB3
QMAX = 65000.0        # quantization full-scale (margin below 65535)

_CACHE = {}


def _bitonic_stages(n):
    stages = []
    p = 1
    while (1 << p) <= n:
        stages.append(("flip", p))
        c = p - 2
        while c >= 0:
            stages.append(("std", c))
            c -= 1
        p += 1
    return stages


# Digit-reversed storage for the sort: logical bit b -> phys weight.
# Digits (logical LSB..MSB): sizes 4,8,8,8 with phys weights 512,64,8,1.
_BITPW = {0: 512, 1: 1024, 2: 64, 3: 128, 4: 256, 5: 8, 6: 16, 7: 32,
          8: 1, 9: 2, 10: 4}
_NBITS = 11


def _merge_dims(entries):
    dims = []
    for step, cnt in entries:
        if dims and dims[-1][0] == step * 2 and (dims[-1][0] > 0) == (step > 0):
            dims[-1] = [step, dims[-1][1] * 2]
            continue
        dims.append([step, cnt])
    return dims


def _stage_ops(kind, param):
    """List of (offA, dimsA, offB, dimsB) op tuples, each with <=3 free dims."""
    if kind == "std":
        c, negset = param, set()
    else:
        c = param - 1
        negset = set(range(c))

    def build(fixed):
        order = sorted((b for b in range(_NBITS) if b != c and b not in fixed),
                       key=lambda b: -_BITPW[b])
        offA = sum(_BITPW[b] * v for b, v in fixed.items())
        offB = _BITPW[c] + offA
        entsA, entsB = [], []
        for b in order:
            pw = _BITPW[b]
            entsA.append((pw, 2))
            if b in negset:
                entsB.append((-pw, 2))
                offB += pw
            else:
                entsB.append((pw, 2))
        return offA, _merge_dims(entsA), offB, _merge_dims(entsB)

    offA, dA, offB, dB = build({})
    if len(dA) <= 3 and len(dB) <= 3:
        return [(offA, dA, offB, dB)]
    t = c + 1
    out = []
    for v in (0, 1):
        o = build({t: v})
        assert len(o[1]) <= 3 and len(o[3]) <= 3, (kind, param, o)
        out.append(o)
    return out


def _emit_sort(nc, bufA, bufB):
    """Sort partition rows of bufA ([128, L] u16 AP) in digit-reversed phys
    layout; sorted ascending when read through the logical-order AP."""
    cur, oth = bufA, bufB
    stages = _bitonic_stages(L)
    assert len(stages) % 2 == 0
    for kind, prm in stages:
        for offA, dA, offB, dB in _stage_ops(kind, prm):
            A_in = bass.AP(tensor=cur.tensor, offset=cur.offset + offA,
                           ap=[cur.ap[0]] + dA)
            B_in = bass.AP(tensor=cur.tensor, offset=cur.offset + offB,
                           ap=[cur.ap[0]] + dB)
            A_out = bass.AP(tensor=oth.tensor, offset=oth.offset + offA,
                            ap=[oth.ap[0]] + dA)
            B_out = bass.AP(tensor=oth.tensor, offset=oth.offset + offB,
                            ap=[oth.ap[0]] + dB)
            nc.vector.tensor_tensor(out=A_out, in0=A_in, in1=B_in, op=OP.min)
            nc.vector.tensor_tensor(out=B_out, in0=A_in, in1=B_in, op=OP.max)
        cur, oth = oth, cur
    assert cur is bufA


def _build_nc():
    nc = bacc.Bacc("TRN2", target_bir_lowering=False, debug=False)

    xT = nc.dram_tensor("xT", [D, L], F32R, kind="ExternalInput")
    wv = nc.dram_tensor("wv", [D, H], F32R, kind="ExternalInput")
    wm = nc.dram_tensor("wm", [D, 3 * D], F32R, kind="ExternalInput")
    wh = nc.dram_tensor("wh", [H, D], F32R, kind="ExternalInput")
    vb = nc.dram_tensor("vb", [D], F32, kind="ExternalInput")
    ub = nc.dram_tensor("ub", [D], F32, kind="ExternalInput")
    rb = nc.dram_tensor("rb", [D], F32, kind="ExternalInput")
    hxb = nc.dram_tensor("hxb", [D], F32, kind="ExternalInput")
    identd = nc.dram_tensor("identd", [128, 128], F32R, kind="ExternalInput")
    # EMA tables: qp[d,n,j]=q^(j+1) j=0..2; q4[d,n]=q^4; cw[d,n,k]=w*q^(k+1);
    # kf[d,j]=sum_n w*q^j (+omega at j=0)
    qp = nc.dram_tensor("qp", [D, N, 3], F32, kind="ExternalInput")
    q4 = nc.dram_tensor("q4", [D, N], F32, kind="ExternalInput")
    cw = nc.dram_tensor("cw", [D, N, 4], F32, kind="ExternalInput")
    kf = nc.dram_tensor("kf", [D, 4], F32, kind="ExternalInput")
    cdesc = nc.dram_tensor("cdesc", [D], F32, kind="ExternalInput")
    y = nc.dram_tensor("y", [D, L], F32, kind="ExternalOutput")

    def gp(t):  # [D, ...] DRAM -> [128 part, G, ...] view
        return t.ap().rearrange("(g p) r -> p g r", p=128) if len(t.shape) == 2 else \
               t.ap().rearrange("(g p) -> p g", p=128)

    with tile.TileContext(nc) as tc, ExitStack() as root:
        dram = root.enter_context(tc.tile_pool(name="dram", bufs=1, space="DRAM"))
        mx_d = dram.tile([D, L], F32R)
        u_d = dram.tile([D, L], F32)
        r_d = dram.tile([D, L], F32)
        hx_d = dram.tile([D, L], F32R)

        persist = root.enter_context(tc.tile_pool(name="persist", bufs=1))
        x_sb = persist.tile([128, G, L], F32R)
        keys = persist.tile([128, G, L], U16)
        prm = persist.tile([128, 12, G], F32)   # [part, param, group]
        ident = persist.tile([128, 128], F32R)
        sortp = root.enter_context(tc.tile_pool(name="sortb", bufs=2))

        nc.sync.dma_start(out=ident, in_=identd.ap())
        qp_sb = persist.tile([128, G, N, 3], F32)
        q4_sb = persist.tile([128, G, N], F32)
        cw_sb = persist.tile([128, G, N, 4], F32)
        kf_sb = persist.tile([128, G, 4], F32)
        nc.sync.dma_start(out=qp_sb, in_=qp.ap().rearrange("(g p) n j -> p g n j", p=128))
        nc.sync.dma_start(out=q4_sb, in_=q4.ap().rearrange("(g p) n -> p g n", p=128))
        nc.sync.dma_start(out=cw_sb, in_=cw.ap().rearrange("(g p) n k -> p g n k", p=128))
        nc.sync.dma_start(out=kf_sb, in_=kf.ap().rearrange("(g p) j -> p g j", p=128))
        nc.sync.dma_start(out=prm[:, 0, :], in_=gp(vb))
        nc.sync.dma_start(out=prm[:, 1, :], in_=gp(ub))
        nc.sync.dma_start(out=prm[:, 2, :], in_=gp(rb))
        nc.sync.dma_start(out=prm[:, 3, :], in_=gp(hxb))
        nc.sync.dma_start(out=prm[:, 4, :], in_=gp(cdesc))
        for g in range(G):
            nc.sync.dma_start(out=x_sb[:, g, :],
                              in_=xT.ap()[g * 128:(g + 1) * 128, :])

        # ------- P2+P1 interleaved: per group scans/conv/mx then vproj/quant -------
        with ExitStack() as p12:
            wvp = p12.enter_context(tc.tile_pool(name="wv", bufs=1))
            wv_sb = wvp.tile([128, G, H], F32R)
            nc.sync.dma_start(out=wv_sb, in_=gp(wv))
            dpool = p12.enter_context(tc.tile_pool(name="diag", bufs=8))
            spool = p12.enter_context(tc.tile_pool(name="scan", bufs=17))
            mpool = p12.enter_context(tc.tile_pool(name="mxe", bufs=2))
            vpool = p12.enter_context(tc.tile_pool(name="v", bufs=2))
            xppool = p12.enter_context(tc.tile_pool(name="xp", bufs=2))
            cps = p12.enter_context(tc.tile_pool(name="cps", bufs=1, space="PSUM"))
            zpool = p12.enter_context(tc.tile_pool(name="zps", bufs=3, space="PSUM"))
            vps = p12.enter_context(tc.tile_pool(name="vps", bufs=1, space="PSUM"))
            for g in range(G):
                # --- EMA: C=4 two-level scan, polyphase PSUM layout ---
                # xp[tau][t] = x[4t+tau], tau-major [128, 4, 512]
                xp = xppool.tile([128, 4, 512], F32R, tag="xp")
                for tau in range(4):
                    xin = x_sb[:, g, :]
                    nc.vector.tensor_copy(
                        out=xp[:, tau, :],
                        in_=bass.AP(tensor=xin.tensor, offset=xin.offset + tau,
                                    ap=[xin.ap[0], [4, 512]]).bitcast(F32))
                # per-basis: all z (PE) + block scans (DVE) first, then all
                # corrections (PE) -- keeps PE ahead of DVE
                s_tiles = []
                for n in range(N):
                    zps = zpool.tile([128, 512], F32, tag="z")
                    for j in range(4):
                        if j == 0:
                            dg = ident
                        else:
                            dg = dpool.tile([128, 128], F32R, tag="dg")
                            nc.scalar.activation(out=dg, in_=ident.bitcast(F32),
                                                 func=AF.Copy,
                                                 scale=qp_sb[:, g, n, j - 1:j])
                        nc.tensor.matmul(out=zps, lhsT=dg, rhs=xp[:, 3 - j, :],
                                         start=(j == 0), stop=(j == 3))
                    # s_t[t] = S[t-1] (shifted block states; s_t[0] = 0)
                    s_t = spool.tile([128, 512], F32R, tag="s")
                    nc.scalar.activation(out=s_t[:, 0:1], in_=prm[:, 0, 0:1],
                                         func=AF.Copy, scale=0.0)
                    nc.vector.tensor_tensor_scan(
                        out=s_t[:, 1:512],
                        data0=q4_sb[:, g, n:n + 1].to_broadcast([128, 511]),
                        data1=zps[:, 0:511], initial=0.0, op0=OP.mult, op1=OP.add)
                    s_tiles.append(s_t)
                # vproj for this group
                v_g = vpool.tile([128, L], F32, tag="v")
                for lb in range(NLB):
                    ps = vps.tile([128, LB], F32)
                    for k in range(G):
                        nc.tensor.matmul(
                            out=ps,
                            lhsT=wv_sb[:, k, g * 128:(g + 1) * 128],
                            rhs=x_sb[:, k, lb * LB:(lb + 1) * LB],
                            start=(k == 0), stop=(k == G - 1))
                    nc.scalar.activation(out=v_g[:, lb * LB:(lb + 1) * LB], in_=ps,
                                         func=AF.Silu, bias=prm[:, 0, g:g + 1], scale=1.0)
                # quantization params + quantize
                nc.vector.memset(prm[:, 5, g:g + 1], -0.279)
                nc.vector.tensor_reduce(out=prm[:, 6, g:g + 1], in_=v_g,
                                        axis=mybir.AxisListType.X, op=OP.max)
                nc.vector.tensor_tensor(out=prm[:, 7, g:g + 1], in0=prm[:, 6, g:g + 1],
                                        in1=prm[:, 5, g:g + 1], op=OP.subtract)
                nc.vector.tensor_scalar_max(prm[:, 7, g:g + 1], prm[:, 7, g:g + 1], 1e-30)
                nc.vector.reciprocal(out=prm[:, 8, g:g + 1], in_=prm[:, 7, g:g + 1])
                nc.vector.tensor_scalar_mul(prm[:, 8, g:g + 1], prm[:, 8, g:g + 1], QMAX)
                nc.vector.scalar_tensor_tensor(out=prm[:, 9, g:g + 1], in0=prm[:, 4, g:g + 1],
                                               scalar=-2.0, in1=prm[:, 8, g:g + 1],
                                               op0=OP.mult, op1=OP.bypass)
                nc.vector.tensor_scalar_add(prm[:, 9, g:g + 1], prm[:, 9, g:g + 1], 1.0)
                nc.vector.tensor_tensor(out=prm[:, 9, g:g + 1], in0=prm[:, 9, g:g + 1],
                                        in1=prm[:, 8, g:g + 1], op=OP.mult)
                nc.vector.tensor_tensor(out=prm[:, 10, g:g + 1], in0=prm[:, 5, g:g + 1],
                                        in1=prm[:, 9, g:g + 1], op=OP.mult)
                nc.vector.scalar_tensor_tensor(out=prm[:, 10, g:g + 1], in0=prm[:, 4, g:g + 1],
                                               scalar=QMAX, in1=prm[:, 10, g:g + 1],
                                               op0=OP.mult, op1=OP.subtract)
                nc.scalar.activation(out=keys[:, g, :], in_=v_g, func=AF.Identity,
                                     scale=prm[:, 9, g:g + 1], bias=prm[:, 10, g:g + 1])
                nc.vector.reciprocal(out=prm[:, 11, g:g + 1], in_=prm[:, 9, g:g + 1])
                nc.vector.scalar_tensor_tensor(out=prm[:, 10, g:g + 1], in0=prm[:, 10, g:g + 1],
                                               scalar=-1.0, in1=prm[:, 11, g:g + 1],
                                               op0=OP.mult, op1=OP.mult)

                conv = cps.tile([128, 4, 512], F32)   # conv_p[k][t] = conv[4t+k]
                # within-block FIR: conv_p[k] += sum_{j<=k} diag(kf[j]) xp[k-j]
                kfd = {}
                for j in range(4):
                    dg = dpool.tile([128, 128], F32R, tag="dg")
                    nc.scalar.activation(out=dg, in_=ident.bitcast(F32), func=AF.Copy,
                                         scale=kf_sb[:, g, j:j + 1])
                    kfd[j] = dg
                for k in range(4):
                    for j in range(k + 1):
                        nc.tensor.matmul(out=conv[:, k, :], lhsT=kfd[j],
                                         rhs=xp[:, k - j, :],
                                         start=(j == 0), stop=False)
                for n in range(N):
                    for k in range(4):
                        dg = dpool.tile([128, 128], F32R, tag="dg")
                        nc.scalar.activation(out=dg, in_=ident.bitcast(F32),
                                             func=AF.Copy,
                                             scale=cw_sb[:, g, n, k:k + 1])
                        nc.tensor.matmul(out=conv[:, k, :], lhsT=dg,
                                         rhs=s_tiles[n],
                                         start=False, stop=(n == N - 1))
                # mx = silu(conv_p) scattered back to natural l order
                mxe = mpool.tile([128, L], F32R, tag="mxe")
                for k in range(4):
                    mo = bass.AP(tensor=mxe.tensor, offset=mxe.offset + k,
                                 ap=[mxe.ap[0], [4, 512]])
                    nc.scalar.activation(out=mo, in_=conv[:, k, :], func=AF.Silu)
                nc.sync.dma_start(out=mx_d[g * 128:(g + 1) * 128, :], in_=mxe)
        # ------- P3a: mxproj -> u/r/hx -> DRAM (emitted before sort: PE overlaps it) ----
        with ExitStack() as p3a:
            wmp = p3a.enter_context(tc.tile_pool(name="wm", bufs=1))
            wm_sb = wmp.tile([128, G, 3 * D], F32R)
            nc.sync.dma_start(out=wm_sb, in_=gp(wm))
            mxi = p3a.enter_context(tc.tile_pool(name="mxi", bufs=2))
            ev = p3a.enter_context(tc.tile_pool(name="ev", bufs=4))
            mps = p3a.enter_context(tc.tile_pool(name="mps", bufs=4, space="PSUM"))
            outmap = [(u_d, AF.Sigmoid, 1, F32), (r_d, AF.Silu, 2, F32),
                      (hx_d, AF.Identity, 3, F32R)]
            for lb in range(NLB):
                mx_sl = mxi.tile([128, G, LB], F32R, tag="mxi")
                nc.sync.dma_start(
                    out=mx_sl,
                    in_=mx_d[:, lb * LB:(lb + 1) * LB].rearrange(
                        "(g p) l -> p g l", p=128))
                for t, (dst, fn, bcol, edt) in enumerate(outmap):
                    for g in range(G):
                        o = t * G + g
                        ps = mps.tile([128, LB], F32)
                        for k in range(G):
                            nc.tensor.matmul(
                                out=ps,
                                lhsT=wm_sb[:, k, o * 128:(o + 1) * 128],
                                rhs=mx_sl[:, k, :],
                                start=(k == 0), stop=(k == G - 1))
                        e = ev.tile([128, LB], edt, tag="ev")
                        nc.scalar.activation(out=e, in_=ps, func=fn,
                                             bias=prm[:, bcol, g:g + 1], scale=1.0)
                        nc.sync.dma_start(
                            out=dst[g * 128:(g + 1) * 128, lb * LB:(lb + 1) * LB],
                            in_=e)

        # ------- Sort (DVE-serial; PE runs P3a concurrently) -------
        for g in range(G):
            scratch = sortp.tile([128, L], U16, tag="sc")
            _emit_sort(nc, keys[:, g, :], scratch[:, :])

        # ------- P3b: dequant, t1, hproj(+hx via identity), h, y -------
        with ExitStack() as p3b:
            whp = p3b.enter_context(tc.tile_pool(name="wh", bufs=1))
            wh_sb = whp.tile([128, G, D], F32R)
            nc.sync.dma_start(out=wh_sb, in_=gp(wh))
            inp = p3b.enter_context(tc.tile_pool(name="p3in", bufs=3))
            t1p = p3b.enter_context(tc.tile_pool(name="t1", bufs=3))
            hp = p3b.enter_context(tc.tile_pool(name="h", bufs=4))
            hps = p3b.enter_context(tc.tile_pool(name="hps", bufs=2, space="PSUM"))
            for lb in range(NLB3):
                sl = slice(lb * LB3, (lb + 1) * LB3)
                u_sl = inp.tile([128, G, LB3], F32, tag="u")
                r_sl = inp.tile([128, G, LB3], F32, tag="r")
                hx_sl = inp.tile([128, G, LB3], F32R, tag="hx")
                for dst, src in ((u_sl, u_d), (r_sl, r_d), (hx_sl, hx_d)):
                    nc.sync.dma_start(
                        out=dst, in_=src[:, sl].rearrange("(g p) l -> p g l", p=128))
                t1 = t1p.tile([128, G, LB3], F32R, tag="t1")
                for g in range(G):
                    kg = keys[:, g, :]
                    kperm = bass.AP(tensor=kg.tensor, offset=kg.offset + lb,
                                    ap=[kg.ap[0], [8, 8], [64, 8], [512, 4]])
                    tout = t1[:, g, :].rearrange("p (a b c) -> p a b c", a=8, b=8, c=4)
                    nc.scalar.activation(out=tout, in_=kperm,
                                         func=AF.Identity, scale=prm[:, 11, g:g + 1],
                                         bias=prm[:, 10, g:g + 1])
                    nc.vector.tensor_tensor(out=t1[:, g, :], in0=t1[:, g, :].bitcast(F32),
                                            in1=r_sl[:, g, :], op=OP.mult)
                ps = hps.tile([128, G, LB3], F32)
                for g in range(G):
                    for k in range(G):
                        nc.tensor.matmul(
                            out=ps[:, g, :],
                            lhsT=wh_sb[:, k, g * 128:(g + 1) * 128],
                            rhs=t1[:, k, :],
                            start=(k == 0), stop=False)
                    nc.tensor.matmul(out=ps[:, g, :], lhsT=ident,
                                     rhs=hx_sl[:, g, :], start=False, stop=True)
                h_t = hp.tile([128, G, LB3], F32, tag="h")
                nc.scalar.activation(out=h_t, in_=ps, func=AF.Silu)
                # y = u*(h - x) + x, batched across groups
                xsl = x_sb[:, :, sl].bitcast(F32)
                nc.vector.tensor_tensor(out=h_t, in0=h_t, in1=xsl, op=OP.subtract)
                nc.vector.tensor_tensor(out=h_t, in0=h_t, in1=u_sl, op=OP.mult)
                nc.vector.tensor_tensor(out=h_t, in0=h_t, in1=xsl, op=OP.add)
                nc.sync.dma_start(
                    out=y.ap().rearrange("(g p) l -> p g l", p=128)[:, :, sl],
                    in_=h_t)

    nc.finalize()
    return nc


def _host_prep(inputs):
    ZD = 192
    x = np.asarray(inputs["x"], np.float32)
    delta = np.asarray(inputs["delta"], np.float32)[..., 0]
    alpha = np.asarray(inputs["alpha"], np.float32)[..., 0]
    beta = np.asarray(inputs["beta"], np.float32)[..., 0]
    gamma = np.asarray(inputs["gamma"], np.float32)
    omega = np.asarray(inputs["omega"], np.float32)
    p = 1.0 / (1.0 + np.exp(-delta.astype(np.float64)))
    q = (1.0 - p / (1.0 + np.exp(-alpha.astype(np.float64)))).astype(np.float32)
    wn = (p * beta * gamma / np.sqrt(N)).astype(np.float32)

    mw = np.asarray(inputs["mxproj_w"], np.float32)
    mb = np.asarray(inputs["mxproj_b"], np.float32)
    wm = np.concatenate([mw[0:D], mw[D + ZD:D + ZD + H], mw[D + ZD + H:]], 0)

    eye = np.eye(128, dtype=np.float32)
    qq = q.astype(np.float64)
    qp = np.stack([qq, qq ** 2, qq ** 3], axis=-1).astype(np.float32)   # [D,N,3]
    q4 = (qq ** 4).astype(np.float32)                                    # [D,N]
    cw = np.stack([wn * (qq ** (k + 1)).astype(np.float32) for k in range(4)],
                  axis=-1).astype(np.float32)                            # [D,N,4]
    kf = np.stack([(wn * (qq ** j).astype(np.float32)).sum(1) for j in range(4)],
                  axis=-1).astype(np.float32)                            # [D,4]
    kf[:, 0] += omega

    shared = dict(
        wv=np.ascontiguousarray(np.asarray(inputs["vproj_w"], np.float32).T),
        wm=np.ascontiguousarray(wm.T),
        wh=np.ascontiguousarray(np.asarray(inputs["hproj_w"], np.float32).T),
        vb=np.asarray(inputs["vproj_b"], np.float32),
        ub=mb[0:D].copy(),
        rb=mb[D + ZD:D + ZD + H].copy(),
        hxb=(mb[D + ZD + H:] + np.asarray(inputs["hproj_b"], np.float32)),
        identd=eye, qp=qp, q4=q4, cw=cw, kf=kf,
        cdesc=np.asarray(inputs["col_descend"]).astype(np.float32),
    )
    xT = np.ascontiguousarray(x.transpose(0, 2, 1))
    return shared, xT


def kernel(**inputs):
    if "nc" not in _CACHE:
        _CACHE["nc"] = _build_nc()
    nc = _CACHE["nc"]
    shared, xT = _host_prep(inputs)
    B = xT.shape[0]
    in_maps = [dict(shared, xT=np.ascontiguousarray(xT[b])) for b in range(B)]
    res = bass_utils.run_bass_kernel_spmd(
        nc, in_maps, core_ids=list(range(B)),
        trace=bool(_CACHE.get("trace", False)))
    _CACHE["last_result"] = res
    out = np.stack([res.results[b]["y"].reshape(D, L).T for b in range(B)])
    return np.ascontiguousarray(out.astype(np.float32))

